# revision 54
# baseline (speedup 1.0000x reference)
"""Trainium2 Bass kernel for nn_MultiHeadAttention_10960756539999.

MHA: inp [2, 2048, 768], 12 heads, head_dim 64, Wqkv [768, 2304] (per-head
192-col slabs laid out [Q|K|V]), Wproj [768, 768].

Sharding: 24 (batch, head) pairs -> 3 heads per core; cores 0-3 take batch 0,
cores 4-7 take batch 1. Each core computes QKV^T for its heads from x^T,
attention fully on-chip (softmax over the free axis of scores^T, no max
subtraction -- scores are ~N(0,1)), and a row-sharded partial projection
out_heads @ Wproj[rows]. The host sums the 4 per-batch partials and adds
bproj.

All tensors are bf16 in DRAM/SBUF (f32 PSUM accumulation): 1 cycle/row on
the PE like fp32r, but half the DMA bytes and SBUF. The schedule is built
around the two engine walls -- PE ~115us of matmul rows and ACT ~100us of
exp -- with every non-attention matmul (V staging, heads 1-2 QK^T, the
projection) broken into single-PSUM-bank "filler units" that are emitted
inside the exp-bound attention j-loops, so the PE chews them while waiting
on exp. Pass order is head-major with h2 last; proj of the first query
half runs as filler in the last pass, proj of the second half is the tail.

Softmax denominators ride row 64 of the attV accumulator (ones column
appended to V); normalization defers to a per-query column scale after
attV: DVE reciprocal (pre-rounded to f32r), a contraction-1 PE matmul
against a ones column to broadcast it across partitions, and a DVE
multiply into the bf16 oT slab. Each pass's normalize is emitted at the
next pass's head so it overlaps the scores prologue.
"""

import os
import sys

import numpy as np
import ml_dtypes

try:
    import concourse.bass as bass
except ImportError:  # harness runs from a bare directory
    sys.path.insert(0, "/opt/trn_rl_repo")
    import concourse.bass as bass

import concourse.tile as tile
from concourse import bacc, mybir
from concourse.bass_utils import run_bass_kernel_spmd

F32 = mybir.dt.float32
BF16 = mybir.dt.bfloat16
AF = mybir.ActivationFunctionType
NPBF16 = ml_dtypes.bfloat16

NH = 12          # total heads
D = 64           # head dim
S = 2048         # sequence length
NI = 768         # model dim
NB = 2           # batch
NCORES = 8
HPC = 3          # heads per core
CPB = NCORES // NB   # cores per batch
KC = NI // 128   # contraction chunks for the 768 dim
NT = S // 128    # 128-row tiles along tokens/keys
HB = S // 2      # 1024: half the token/query axis
SCALE = float(1.0 / np.sqrt(NI / NH))  # 1/8

# filled by kernel() for test.py to report
last_results = None

_cache = {}


def _build_nc(has_bias: bool):
    nc = bacc.Bacc("TRN2", target_bir_lowering=False, debug=False,
                   num_devices=NCORES)

    xT_d = nc.dram_tensor("xT", [NI, S], BF16, kind="ExternalInput")
    wqk_d = nc.dram_tensor("wqk", [NI, HPC * 128], BF16, kind="ExternalInput")
    # wv padded to 256 cols so each DMA descriptor is 512B (full-rate)
    wv_d = nc.dram_tensor("wv", [NI, 256], BF16, kind="ExternalInput")
    wp_d = nc.dram_tensor("wp", [HPC * D, NI], BF16, kind="ExternalInput")
    if has_bias:
        # cols 2h = bq_h, 2h+1 = bk_h (64 rows each); bv packed per-head
        bqk_d = nc.dram_tensor("bqk", [D, 2 * HPC], F32, kind="ExternalInput")
        bv_d = nc.dram_tensor("bv", [HPC * D], F32, kind="ExternalInput")
    out_d = nc.dram_tensor("out", [S, NI], BF16, kind="ExternalOutput")

    with tile.TileContext(nc) as tc:
        with (
            tc.tile_pool(name="const", bufs=1) as constp,
            tc.tile_pool(name="expp", bufs=6) as expp,
            tc.tile_pool(name="opool", bufs=1) as opool,
            tc.tile_pool(name="rwork", bufs=2) as rwork,
            # PSUM (8 banks x 2KB): tag A = 2 slots x 2 banks (QK^T h0 in
            # the load phase, then the 2-deep scores rotation, then proj
            # tail), tag B = 1 slot x 2 banks (V group 0, then per-pass attV
            # accumulator), tag C = 1 slot x 2 banks (V group 1, then the
            # in-pass filler units).
            tc.tile_pool(name="ps", bufs=2, space="PSUM") as psp,
            tc.tile_pool(name="dramp", bufs=2, space="DRAM") as dramp,
        ):
            # ---- SBUF tensors ----
            xT = constp.tile([128, KC, S], BF16, tag="xT")
            wqk = constp.tile([128, KC, HPC * 128], BF16, tag="wqk")
            wv = constp.tile([128, KC, 256], BF16, tag="wv")
            wp01 = constp.tile([128, NI], BF16, tag="wp01")
            wp2 = constp.tile([D, NI], BF16, tag="wp2")
            qq = [constp.tile([D, S], BF16, tag=f"qq{h}", name=f"qq{h}")
                  for h in range(HPC)]
            kk = [constp.tile([D, S], BF16, tag=f"kk{h}", name=f"kk{h}")
                  for h in range(HPC)]
            # vaug layout: [128, h, NT*65]; per key-chunk j the slab
            # [:, h, 65j : 65j+65] is [V_h(chunk j) | ones].
            vaug = constp.tile([128, HPC, NT * 65], BF16, tag="vaug")
            oT01 = opool.tile([128, S], BF16, tag="oT01")
            oT2 = opool.tile([D, S], BF16, tag="oT2")

            # ---- input DMAs: first chunk, then wv (small), then the rest
            # of the wqk/xT stream, then the proj weights ----
            xT_src = xT_d[:].rearrange("(c p) s -> p c s", p=128)
            wqk_src = wqk_d[:].rearrange("(c p) m -> p c m", p=128)
            wv_src = wv_d[:].rearrange("(c p) m -> p c m", p=128)
            for c in range(KC):
                nc.sync.dma_start(out=wqk[:, c, :], in_=wqk_src[:, c, :])
                nc.sync.dma_start(out=xT[:, c, :], in_=xT_src[:, c, :])
            for c in range(KC):
                nc.sync.dma_start(out=wv[:, c, :], in_=wv_src[:, c, :])
            nc.sync.dma_start(out=wp01, in_=wp_d[0:128, :])
            nc.sync.dma_start(out=wp2, in_=wp_d[128:HPC * D, :])
            if has_bias:
                bqk = constp.tile([D, 2 * HPC], F32, tag="bqk")
                nc.sync.dma_start(out=bqk, in_=bqk_d[:])
                bvb = constp.tile([128, HPC * D], F32, tag="bvb")
                bv_ap = bv_d[:]
                bv_bcast = bass.AP(
                    tensor=bv_ap.tensor, offset=bv_ap.offset,
                    ap=[[0, 128]] + [list(p) for p in bv_ap.ap])
                nc.sync.dma_start(out=bvb, in_=bv_bcast)
                bvb3 = bvb.rearrange("p (h d) -> p h d", d=D)

            # ---- ACT warmup: trigger the Exp table load at t=0 ----
            warm_in = constp.tile([1, 2], F32, tag="warm_in")
            warm_out = constp.tile([1, 2], F32, tag="warm_out")
            nc.vector.memset(warm_in, 0.0)
            nc.scalar.activation(warm_out, warm_in, AF.Exp, scale=1.0)

            # PE-warmup scratch first in the DVE queue so warm matmuls
            # start immediately
            scr = constp.tile([128, 512], BF16, tag="scr")
            nc.vector.memset(scr, 0.0)

            # ones columns of vaug
            ones_sb = constp.tile([128, NT, 1], BF16, tag="ones")
            nc.vector.memset(ones_sb, 1.0)
            v4 = vaug.rearrange("p h (t c) -> p h t c", c=65)
            for h in range(HPC):
                nc.vector.tensor_copy(v4[:, h, :, 64:65], ones_sb)

            # ones column for the reciprocal broadcast matmul (f32r: PE
            # inputs must be produced pre-rounded; memset can't write f32r
            # so round via a DVE copy)
            F32R = mybir.dt.float32r
            ones_rf = constp.tile([1, D], F32, tag="ones_rf")
            nc.vector.memset(ones_rf, 1.0)
            ones_r = constp.tile([1, D], F32R, tag="ones_r")
            with nc.allow_low_precision(
                    reason="f32r pre-round of the broadcast ones column"):
                nc.vector.tensor_copy(ones_r, ones_rf)

            # ---- copy helpers (PSUM sources: DVE or ACT only --
            # GPSIMD cannot access PSUM) ----
            def qk_copy(h, sl, src, k_eng="v"):
                # src [128, len] psum (q rows 0:64, k rows 64:128); the k
                # copy shifts partitions 64-127 down to 0-63
                if has_bias:
                    nc.vector.tensor_scalar_add(
                        qq[h][:, sl], src[0:D, :], bqk[:, 2 * h:2 * h + 1])
                    nc.vector.tensor_scalar_add(
                        kk[h][:, sl], src[D:128, :],
                        bqk[:, 2 * h + 1:2 * h + 2])
                    return
                nc.vector.tensor_copy(qq[h][:, sl], src[0:D, :])
                if k_eng == "a":
                    nc.scalar.copy(kk[h][:, sl], src[D:128, :])
                else:
                    nc.vector.tensor_copy(kk[h][:, sl], src[D:128, :])

            def v_copy(pv, ti, t):
                # pv [128, n, 256] psum; one fused copy into all 3 heads'
                # vaug slabs
                dst = v4[:, :, t, 0:D]
                src = pv[:, ti, 0:HPC * D].rearrange("p (h d) -> p h d", d=D)
                if has_bias:
                    nc.vector.tensor_add(dst, src, bvb3)
                else:
                    nc.vector.tensor_copy(dst, src)

            # ---- PE p-state warmup into the bank the first real
            # start=True matmul will clear ----
            qktt = [psp.tile([128, HB], F32, tag="A", name=f"qkt0_{half}")
                    for half in range(2)]
            for _ in range(6):
                nc.tensor.matmul(qktt[0][:, 0:512], scr[:, 0:128], scr,
                                 start=True, stop=True,
                                 skip_group_check=True)

            # ---- load phase: QK^T(h0) tracks the xT chunk stream; the
            # V matmuls (wv arrives after xT) are emitted inside pass 0
            # between the scores prologue and the j-loop ----
            pv0 = psp.tile([128, 4, 256], F32, tag="B", name="pv0", bufs=1)
            pv1 = psp.tile([128, 2, 256], F32, tag="C", name="pv1", bufs=1)
            pv2 = psp.tile([128, 2, 256], F32, tag="D", name="pv2", bufs=1)
            for c in range(KC):
                for half in range(2):
                    for n in range(2):
                        sl = slice(half * HB + n * 512,
                                   half * HB + (n + 1) * 512)
                        nc.tensor.matmul(
                            qktt[half][:, n * 512:(n + 1) * 512],
                            wqk[:, c, 0:128], xT[:, c, sl],
                            start=(c == 0), stop=(c == KC - 1),
                            skip_group_check=True)
            # gate sc(0)/sc(1): q copies on DVE, k half0 on the idle ACT
            nc.vector.tensor_copy(qq[0][:, 0:HB], qktt[0][0:D, :])
            nc.scalar.copy(kk[0][:, 0:HB], qktt[0][D:128, :])
            nc.vector.tensor_copy(qq[0][:, HB:S], qktt[1][0:D, :])

            def pv_mms(pv, base):
                for c in range(KC):
                    for ti in range(pv.shape[1]):
                        t = base + ti
                        nc.tensor.matmul(
                            pv[:, ti, 0:HPC * D],
                            xT[:, c, t * 128:(t + 1) * 128],
                            wv[:, c, 0:HPC * D],
                            start=(c == 0 and ti % 2 == 0),
                            stop=(c == KC - 1), skip_group_check=True)

            def p0_preloop():
                # V tiles 0-7: matmuls on the PE while the h0 copies and
                # the first exps run; drains in consumption order
                pv_mms(pv0, 0)
                pv_mms(pv1, 4)
                pv_mms(pv2, 6)
                for ti in range(4):
                    v_copy(pv0, ti, ti)
                for ti in range(2):
                    v_copy(pv1, ti, 4 + ti)
                for ti in range(2):
                    v_copy(pv2, ti, 6 + ti)
                nc.vector.tensor_copy(kk[0][:, HB:S], qktt[1][D:128, :])
                if has_bias:
                    nc.vector.tensor_scalar_add(
                        qq[0][:, :], qq[0][:, :], bqk[:, 0:1])
                    nc.vector.tensor_scalar_add(
                        kk[0][:, :], kk[0][:, :], bqk[:, 1:2])

            def pv_unit(pv, base):
                def emit():
                    pv_mms(pv, base)
                    for ti in range(pv.shape[1]):
                        v_copy(pv, ti, base + ti)
                return emit

            # ---- filler units: each fits a single PSUM bank, and
            # alternates between the C and D slots so one unit's drain
            # copy overlaps the next unit's matmuls ----
            def v_unit(t, tag):
                def emit():
                    pvu = psp.tile([128, 1, 256], F32, tag=tag, bufs=1,
                                   name=f"vu{t}")
                    for c in range(KC):
                        nc.tensor.matmul(
                            pvu[:, 0, 0:HPC * D],
                            xT[:, c, t * 128:(t + 1) * 128],
                            wv[:, c, 0:HPC * D],
                            start=(c == 0), stop=(c == KC - 1))
                    v_copy(pvu, 0, t)
                return emit

            def qkt_piece(h, half, n, tag):
                # three emissions (consecutive filler slots) so the 6-chunk
                # matmul burst doesn't starve the exp stream
                state = {}
                sl = slice(half * HB + n * 512, half * HB + (n + 1) * 512)

                def mm_range(c0, c1):
                    for c in range(c0, c1):
                        nc.tensor.matmul(
                            state["pc"], wqk[:, c, h * 128:(h + 1) * 128],
                            xT[:, c, sl],
                            start=(c == 0), stop=(c == KC - 1))

                def emit_a():
                    state["pc"] = psp.tile([128, 512], F32, tag=tag, bufs=1,
                                           name=f"qp{h}_{half}_{n}")
                    mm_range(0, 2)

                def emit_b():
                    mm_range(2, 4)

                def emit_c():
                    mm_range(4, KC)
                    qk_copy(h, sl, state["pc"])
                return emit_a, emit_b, emit_c

            # ---- projection ----
            out_dst = out_d[:].rearrange("(t p) o -> t p o", p=128)
            ostage = {}

            def ost_for(t):
                if t not in ostage:
                    ostage[t] = rwork.tile([128, NI], BF16, tag="ostage",
                                           bufs=6, name=f"ost{t}")
                return ostage[t]

            def proj_piece(t, r, tag, eng):
                # one output region (r=0: cols 0:512, r=1: 512:768) of proj
                # tile t through a single-bank slot
                def emit():
                    n0, n1 = ((0, 512), (512, NI))[r]
                    pp = psp.tile([128, 512], F32, tag=tag, bufs=1,
                                  name=f"pp{t}_{r}")
                    o01 = oT01[:, t * 128:(t + 1) * 128]
                    o2 = oT2[:, t * 128:(t + 1) * 128]
                    nc.tensor.matmul(pp[:, 0:n1 - n0], o01, wp01[:, n0:n1],
                                     start=True, stop=False)
                    nc.tensor.matmul(pp[:, 0:n1 - n0], o2, wp2[:, n0:n1],
                                     start=False, stop=True)
                    ost = ost_for(t)
                    if eng == "a":
                        nc.scalar.copy(ost[:, n0:n1], pp[:, 0:n1 - n0])
                    else:
                        nc.vector.tensor_copy(ost[:, n0:n1], pp[:, 0:n1 - n0])
                    nc.sync.dma_start(out=out_dst[t][:, n0:n1],
                                      in_=ost[:, n0:n1])
                return emit

            def proj_mm1(t, tag):
                pp = psp.tile([128, HB], F32, tag=tag,
                              bufs=(None if tag == "A" else 1),
                              name=f"pp{t}")
                o01 = oT01[:, t * 128:(t + 1) * 128]
                for n0, n1 in ((0, 512), (512, NI)):
                    nc.tensor.matmul(pp[:, n0:n1], o01, wp01[:, n0:n1],
                                     start=True, stop=False)
                return pp

            def proj_mm2(t, pp, eng):
                o2 = oT2[:, t * 128:(t + 1) * 128]
                for n0, n1 in ((0, 512), (512, NI)):
                    nc.tensor.matmul(pp[:, n0:n1], o2, wp2[:, n0:n1],
                                     start=False, stop=True)
                ost = ost_for(t)
                if eng == "a":
                    nc.scalar.copy(ost, pp[:, 0:NI])
                else:
                    nc.vector.tensor_copy(ost, pp[:, 0:NI])
                nc.sync.dma_start(out=out_dst[t], in_=ost)

            def proj_unit(t, tag, eng):
                def emit():
                    proj_mm2(t, proj_mm1(t, tag), eng)
                return emit

            # ---- attention pass ----
            def attention_pass(h, qh, filler, last=False, pre_loop=None):
                # rows 0:64 = out_h^T unnormalized, row 64 = denominators
                acc = psp.tile([D + 1, HB], F32, tag="B", name=f"acc{h}{qh}",
                               bufs=1)

                def sc_mms(j):
                    sc = psp.tile([128, HB], F32, tag="A", name="sc")
                    klhs = kk[h][:, j * 128:(j + 1) * 128]
                    for n in range(2):
                        sl = slice(qh * HB + n * 512,
                                   qh * HB + (n + 1) * 512)
                        nc.tensor.matmul(
                            sc[:, n * 512:(n + 1) * 512], klhs,
                            qq[h][:, sl])
                    return sc

                sc_q = [sc_mms(0), sc_mms(1)]
                if pre_loop is not None:
                    pre_loop()
                fq = list(filler)
                for j in range(NT):
                    ex = expp.tile([128, HB], BF16, tag="exp")
                    nc.scalar.activation(ex, sc_q.pop(0), AF.Exp,
                                         scale=SCALE)
                    if j + 2 < NT:
                        sc_q.append(sc_mms(j + 2))
                    # filler between the scores matmul and attV so the PE
                    # has work while exp(j) is still running
                    if fq and fq[0][0] <= j:
                        fq.pop(0)[1]()
                    for n in range(2):
                        nc.tensor.matmul(
                            acc[:, n * 512:(n + 1) * 512],
                            vaug[:, h, j * 65:j * 65 + 65],
                            ex[:, n * 512:(n + 1) * 512],
                            start=(j == 0), stop=(j == NT - 1))
                for _, f in fq:  # leftovers (shouldn't happen)
                    f()
                return acc

            def norm(h, qh, acc):
                # normalize: out_h^T[d, q] * (1 / denom[q]). Stage acc
                # rows to SBUF (frees the B bank), take the reciprocal row
                # pre-rounded to f32r, broadcast it across 64 partitions
                # with contraction-1 PE matmuls into the C/D slots, and
                # scale into the bf16 oT slab.
                ustage = rwork.tile([D + 1, HB], F32, tag="ustage", bufs=2)
                nc.vector.tensor_copy(ustage, acc)  # single-op release
                rrow = rwork.tile([1, HB], F32R, tag="rrow", bufs=2)
                with nc.allow_low_precision(
                        reason="f32r pre-round for the PE broadcast matmul"):
                    nc.vector.reciprocal(rrow, ustage[D:D + 1, :])
                if h == 0:
                    dst = oT01[0:D, :]
                elif h == 1:
                    dst = oT01[D:128, :]
                else:
                    dst = oT2
                for half, tag in ((0, "C"), (1, "D")):
                    hsl = slice(half * 512, (half + 1) * 512)
                    rbp = psp.tile([D, 512], F32, tag=tag, bufs=1,
                                   name=f"rbp{h}{qh}{half}")
                    nc.tensor.matmul(rbp, ones_r, rrow[0:1, hsl],
                                     start=True, stop=True)
                    nc.vector.tensor_mul(
                        dst[:, qh * HB + half * 512:
                            qh * HB + (half + 1) * 512],
                        ustage[0:D, hsl], rbp)

            # ---- pass schedule (head-major, h2 last) ----
            # p0 (h0,q0): k(h0) half1 copy + V tiles 8-15
            # p1 (h0,q1): QK^T(h1) pieces
            # p2 (h1,q0): QK^T(h2) pieces 0-1
            # p3 (h1,q1): QK^T(h2) pieces 2-3
            # p4 (h2,q0): --
            # p5 (h2,q1): proj of query half 0 (tiles 0-7)
            # tail: proj of query half 1 (tiles 8-15)
            cd = ("C", "D")

            def pieces(h, entries, j0, dj):
                out = []
                for i, (hf, n) in enumerate(entries):
                    for k, e in enumerate(qkt_piece(h, hf, n, cd[i % 2])):
                        out.append((j0 + dj * i + k, e))
                return out

            p0 = [(4 + i, v_unit(8 + i, cd[i % 2])) for i in range(8)]
            p1 = pieces(1, ((0, 0), (0, 1), (1, 0), (1, 1)), 3, 3)
            p2 = pieces(2, ((0, 0), (0, 1)), 3, 5)
            p3 = pieces(2, ((1, 0), (1, 1)), 3, 5)
            p4 = []
            # last proj tile of the q0 half is emitted in the tail so its
            # drain copies don't queue ahead of the tail reciprocal
            p5 = [(1 + i, proj_piece(i // 2, i % 2, cd[i % 2], "v"))
                  for i in range(14)]
            acc00 = attention_pass(0, 0, p0, pre_loop=p0_preloop)
            acc01 = attention_pass(0, 1, p1,
                                   pre_loop=lambda: norm(0, 0, acc00))
            acc10 = attention_pass(1, 0, p2,
                                   pre_loop=lambda: norm(0, 1, acc01))
            acc11 = attention_pass(1, 1, p3,
                                   pre_loop=lambda: norm(1, 0, acc10))
            acc20 = attention_pass(2, 0, p4,
                                   pre_loop=lambda: norm(1, 1, acc11))
            norm(2, 0, acc20)
            acc_last = attention_pass(2, 1, p5, last=True)

            # ---- tail: normalize h2/q1 and project tiles 8-15. The
            # reciprocal (DVE) and the acc staging copy (ACT) run in
            # parallel right after the last attV; the reciprocal row is
            # partition-broadcast with a contraction-1 PE matmul into the
            # freed B slot; proj tiles stream through A/A/B with copies
            # alternating DVE/ACT.
            pp_pre = {t: proj_mm1(t, "A") for t in (8, 9)}
            rrow_t = rwork.tile([1, HB], F32R, tag="rrow_t", bufs=1)
            with nc.allow_low_precision(
                    reason="f32r pre-round for the PE broadcast matmul"):
                nc.vector.reciprocal(rrow_t, acc_last[D:D + 1, :])
            ustage_t = rwork.tile([D, HB], F32, tag="ustage", bufs=2)
            nc.scalar.copy(ustage_t, acc_last[0:D, :])
            proj_piece(7, 0, "C", "a")()
            proj_piece(7, 1, "D", "a")()
            rbp_t = psp.tile([D, HB], F32, tag="B", bufs=1, name="rbp_t")
            for half in range(2):
                hsl = slice(half * 512, (half + 1) * 512)
                nc.tensor.matmul(rbp_t[:, hsl], ones_r, rrow_t[0:1, hsl],
                                 start=True, stop=True)
                nc.vector.tensor_mul(
                    oT2[:, HB + half * 512:HB + (half + 1) * 512],
                    ustage_t[:, hsl], rbp_t[:, hsl])
            proj_mm2(8, pp_pre[8], "v")
            proj_mm2(9, pp_pre[9], "a")
            for i, t in enumerate(range(10, NT)):
                tag = ("B", "A", "A", "B", "A", "A")[i]
                proj_unit(t, tag, ("v", "a", "v", "a", "v", "a")[i])()

    nc.compile()
    return nc


def _get_nc(has_bias: bool):
    if has_bias not in _cache:
        _cache[has_bias] = _build_nc(has_bias)
    return _cache[has_bias]


def kernel(inp, Wqkv, bqkv, Wproj, bproj):
    global last_results
    inp = np.ascontiguousarray(np.asarray(inp, dtype=np.float32))
    Wqkv = np.asarray(Wqkv, dtype=np.float32)
    bqkv = np.asarray(bqkv, dtype=np.float32)
    Wproj = np.asarray(Wproj, dtype=np.float32)
    bproj = np.asarray(bproj, dtype=np.float32)
    assert inp.shape == (NB, S, NI), inp.shape

    has_bias = bool(np.any(bqkv))
    nc = _get_nc(has_bias)

    xTs = [np.ascontiguousarray(inp[b].T).astype(NPBF16) for b in range(NB)]

    in_maps = []
    for core in range(NCORES):
        b = core // CPB
        heads = [(core % CPB) * HPC + i for i in range(HPC)]
        wqk = np.empty((NI, HPC * 128), np.float32)
        wvm = np.zeros((NI, 256), np.float32)
        wp = np.empty((HPC * D, NI), np.float32)
        for i, h in enumerate(heads):
            base = h * 3 * D
            wqk[:, i * 128:i * 128 + D] = Wqkv[:, base:base + D]
            wqk[:, i * 128 + D:(i + 1) * 128] = Wqkv[:, base + D:base + 2 * D]
            wvm[:, i * D:(i + 1) * D] = Wqkv[:, base + 2 * D:base + 3 * D]
            wp[i * D:(i + 1) * D, :] = Wproj[h * D:(h + 1) * D, :]
        m = {"xT": xTs[b], "wqk": wqk.astype(NPBF16),
             "wv": wvm.astype(NPBF16), "wp": wp.astype(NPBF16)}
        if has_bias:
            bqk = np.empty((D, 2 * HPC), np.float32)
            bv = np.empty((HPC * D,), np.float32)
            for i, h in enumerate(heads):
                base = h * 3 * D
                bqk[:, 2 * i] = bqkv[base:base + D]
                bqk[:, 2 * i + 1] = bqkv[base + D:base + 2 * D]
                bv[i * D:(i + 1) * D] = bqkv[base + 2 * D:base + 3 * D]
            m["bqk"] = bqk
            m["bv"] = bv
        in_maps.append(m)

    res = run_bass_kernel_spmd(nc, in_maps, core_ids=list(range(NCORES)))
    last_results = res

    out = np.zeros((NB, S, NI), np.float32)
    for core in range(NCORES):
        out[core // CPB] += np.asarray(res.results[core]["out"],
                                       dtype=np.float32)
    out += bproj
    return out
